# revision 10
# baseline (speedup 1.0000x reference)
"""Trainium2 Bass kernel: LSTM neighbor-sequence aggregator + projection.

Model (reference): for each node v, run an LSTM (H=256) over the features
(F=128) of the targets of v's outgoing edges (in original edge order), take
the hidden state at the last valid step, concat with v's own features, and
project with W_out ([F+H, OUT]).

Strategy (v3 = baseline pipeline + fp8-DoubleRow wide matmuls)
--------------------------------------------------------------
Host (numpy):
  * Edges sorted by src (stable) -> per-node neighbor id lists; nodes dealt
    round-robin by degree rank onto 8 cores; shared non-increasing schedule
    M_t places each node at a column whose lifetime equals its degree.
  * Wide-phase neighbor features packed as fp8e4 pairs [128, 2, S_wide]:
    slot0 = e4m3(x), slot1 = e4m3(x - slot0) (residual -> x at ~2x mantissa).

Device (identical SPMD program, 8 cores):
  * Wide steps (M_t > 512): per 512-column chunk and gate block, TWO fp8
    DoubleRow matmuls (vs three f32r): W_ih pairs contract (x_hi, x_lo),
    W_hh pairs contract (h0, h1) with h stored as fp8 [128, 2, CH] --
    exactly the DoubleRow rhs layout.  PSUM/ACT/DVE pipeline is the proven
    per-bank structure: one PSUM bank per gate block (8 rotating tiles),
    per-block sigmoid/tanh ACT with the bias operand, DVE cell update.
    Filler matmuls after each chunk keep the PE activity window busy so
    HAM never re-throttles.
  * A bf16 copy of each column's final h is written for the narrow slice
    of columns ending at each step; projections read it (fp8 h would cost
    ~2e-3 extra error).
  * Tail steps (M_t <= 512): unchanged baseline bf16 machinery (packed
    deep-tail banks, delta-pattern bias matmuls, grouped ACTs, fillers).
"""

import math
import os
import sys

for _p in (
    "/opt/trn_rl_repo",
    "/root/.axon_site",
    "/root/.axon_site/_ro/trn_rl_repo",
    "/root/.axon_site/_ro/pypackages",
):
    if os.path.isdir(_p) and _p not in sys.path:
        sys.path.append(_p)

import numpy as np

import concourse.bass as bass
import concourse.tile as tile
from concourse import bacc, mybir
from concourse.bass_utils import run_bass_kernel_spmd

NCORES = 8
F, H, OUT = 128, 256, 256
CH = 512  # chunk width (matmul free dim; one fp32 PSUM bank)

F32 = mybir.dt.float32
BF16 = mybir.dt.bfloat16
FP8 = mybir.dt.float8e4
DRM = mybir.MatmulPerfMode.DoubleRow

_SIG = mybir.ActivationFunctionType.Sigmoid
_TANH = mybir.ActivationFunctionType.Tanh
# gate blocks (PyTorch order i,f,g,o; two 128-row blocks each)
_GATE_FUNC = [_SIG, _SIG, _SIG, _SIG, _TANH, _TANH, _SIG, _SIG]
# emit order: i,g first (DVE needs i*g), then f, then o
_MI_ORDER = [0, 1, 4, 5, 2, 3, 6, 7]


# ---------------------------------------------------------------- host side

def _preprocess(input_matrix, adjacency):
    """Partition nodes, build shared schedule + packed per-core inputs."""
    N = input_matrix.shape[0]
    src, trg = adjacency[0], adjacency[1]

    order = np.argsort(src, kind="stable")
    trg_s = trg[order]
    counts = np.bincount(src, minlength=N).astype(np.int64)
    offsets = np.zeros(N + 1, np.int64)
    np.cumsum(counts, out=offsets[1:])

    rank_order = np.argsort(-counts, kind="stable")
    core_nodes = [rank_order[c::NCORES] for c in range(NCORES)]
    deg_c = [counts[cn] for cn in core_nodes]

    T = int(counts.max())
    cnt = np.zeros((NCORES, T + 1), np.int64)
    for c in range(NCORES):
        h = np.bincount(deg_c[c], minlength=T + 1)
        cs = np.cumsum(h)
        cnt[c, :] = len(deg_c[c]) - cs[: T + 1]
    D = np.max(cnt[:, :-1] - cnt[:, 1:], axis=0)  # D[d-1] for d=1..T
    M = np.zeros(T + 1, np.int64)
    for t in range(T - 1, -1, -1):
        M[t] = -(-(M[t + 1] + D[t]) // 4) * 4

    ALL_COL = int(M[0])
    col_node = []
    deg0 = []
    for c in range(NCORES):
        cn = np.full(ALL_COL, -1, np.int64)
        for d in range(T, 0, -1):
            s0 = int(cnt[c, d])
            k = int(cnt[c, d - 1]) - s0
            if k:
                cn[int(M[d]) : int(M[d]) + k] = core_nodes[c][s0 : s0 + k]
        deg0.append(core_nodes[c][deg_c[c] == 0])  # handled on host
        col_node.append(cn)

    Mt = M[:-1]
    off = np.zeros(T + 1, np.int64)
    np.cumsum(Mt, out=off[1:])
    S = int(off[T])

    TSW = next((t for t in range(1, T) if int(Mt[t]) <= CH), T)
    S_wide = int(off[TSW])
    S_tail = S - S_wide

    e4 = mybir.dt.np(FP8)
    bf = mybir.dt.np(BF16)
    im32 = np.ascontiguousarray(input_matrix, np.float32)
    x_hi = im32.astype(e4)
    x_lo = (im32 - x_hi.astype(np.float32)).astype(e4)

    xseq8, xseqb, xown = [], [], []
    for c in range(NCORES):
        cn = col_node[c]
        xs8 = np.zeros((S_wide, 2, F), e4)
        xs8[:, 1, 0] = 1.0  # bias channel rides slot1 row0
        xsb = np.zeros((S_tail, F), np.float32)
        for t in range(T):
            m = int(Mt[t])
            colnodes = cn[:m]
            valid = colnodes >= 0
            vnodes = colnodes[valid]
            nbr = trg_s[offsets[vnodes] + t]
            if t < TSW:
                o = int(off[t])
                blk = xs8[o : o + m]
                tmp = blk[valid]
                tmp[:, 0, :] = x_hi[nbr]
                tmp[:, 1, 1:] = x_lo[nbr][:, 1:]  # row0 = bias channel
                blk[valid] = tmp
            else:
                o = int(off[t] - S_wide)
                xsb[o : o + m][valid] = im32[nbr]
        xseq8.append(np.ascontiguousarray(xs8.transpose(2, 1, 0)))
        xseqb.append(np.ascontiguousarray(xsb.T.astype(bf)))
        xo = np.zeros((ALL_COL, F), np.float32)
        valid = cn >= 0
        xo[valid] = im32[cn[valid]]
        xown.append(np.ascontiguousarray(xo.T.astype(bf)))

    return dict(T=T, M=Mt, off=off, S=S, AC=ALL_COL, TSW=TSW,
                xseq8=xseq8, xseqb=xseqb, xown=xown,
                col_node=col_node, deg0=deg0)


def _make_in_maps(pp, W_ih, W_hh, b_ih, b_hh, W_out):
    e4 = mybir.dt.np(FP8)
    bf = mybir.dt.np(BF16)
    Wi = np.ascontiguousarray(W_ih).astype(np.float32)   # [4H, F]
    Wh = np.ascontiguousarray(W_hh).astype(np.float32)   # [4H, H]
    bc = (b_ih + b_hh).astype(np.float32)                # [4H]

    Wi8 = Wi.T.astype(e4)                                # [F, 4H]
    wih8 = np.zeros((F, 2, 4 * H), e4)
    wih8[:, 0, :] = Wi8
    wih8[1:, 1, :] = Wi8[1:]     # slot1 pairs the x residual with the same W
    wih8[0, 1, :] = bc.astype(e4)  # ... except row0, which delivers the bias
    whh8 = np.zeros((128, 2, 4 * H), e4)
    whh8[:, 0, :] = Wh.T[0:128].astype(e4)
    whh8[:, 1, :] = Wh.T[128:256].astype(e4)

    wlb = np.stack([
        np.ascontiguousarray(Wi.T),
        np.ascontiguousarray(Wh.T[:128]),
        np.ascontiguousarray(Wh.T[128:]),
    ]).astype(bf)
    wo = np.stack([W_out[0:128], W_out[128:256], W_out[256:384]]).astype(bf)
    bcm = np.ascontiguousarray(bc.reshape(8, 128).T)     # [128, 8]
    bct8 = bcm.T.astype(bf)                              # [8, 128]
    be8 = np.zeros((8, 8, 64), np.float32)
    be8[np.arange(8), np.arange(8), :] = 1.0
    be4 = np.zeros((4, 4, 128), np.float32)
    be4[np.arange(4), np.arange(4), :] = 1.0

    maps = []
    for c in range(NCORES):
        maps.append({
            "xseq8": pp["xseq8"][c], "xseqb": pp["xseqb"][c],
            "xown": pp["xown"][c],
            "wih8": wih8, "whh8": whh8, "wlb": wlb, "wo": wo,
            "bc": bcm, "bct8": bct8,
            "be8": be8.astype(bf), "be4": be4.astype(bf),
        })
    return maps


# ------------------------------------------------------------- bass program

def build_program(T, Mt, off, S, AC, TSW):
    nc = bacc.Bacc("TRN2", target_bir_lowering=False, debug=False,
                   enable_asserts=False)

    S_wide = int(off[TSW])
    S_tail = int(off[T]) - S_wide

    xseq8_d = nc.declare_dram_parameter("xseq8", [128, 2, S_wide], FP8,
                                        isOutput=False)
    xseqb_d = nc.declare_dram_parameter("xseqb", [128, max(S_tail, 1)], BF16,
                                        isOutput=False)
    xown_d = nc.declare_dram_parameter("xown", [128, AC], BF16, isOutput=False)
    wih8_d = nc.declare_dram_parameter("wih8", [128, 2, 1024], FP8,
                                       isOutput=False)
    whh8_d = nc.declare_dram_parameter("whh8", [128, 2, 1024], FP8,
                                       isOutput=False)
    wlb_d = nc.declare_dram_parameter("wlb", [3, 128, 1024], BF16,
                                      isOutput=False)
    wo_d = nc.declare_dram_parameter("wo", [3, 128, 256], BF16, isOutput=False)
    bc_d = nc.declare_dram_parameter("bc", [128, 8], F32, isOutput=False)
    bct8_d = nc.declare_dram_parameter("bct8", [8, 128], BF16, isOutput=False)
    be8_d = nc.declare_dram_parameter("be8", [8, 8, 64], BF16, isOutput=False)
    be4_d = nc.declare_dram_parameter("be4", [4, 4, 128], BF16, isOutput=False)
    out_d = nc.declare_dram_parameter("out", [2, 128, AC], F32, isOutput=True)

    NCH = math.ceil(AC / CH)
    last_touch = [max(t for t in range(T) if Mt[t] > j * CH)
                  for j in range(NCH)]
    proj_at = [T - 1] + [min(max(last_touch[j], TSW + 3 * j), T - 2)
                         for j in range(1, NCH)]

    with tile.TileContext(nc) as tc:
        with (
            tc.tile_pool(name="const", bufs=1) as constp,
            tc.tile_pool(name="state", bufs=1) as statep,
            tc.tile_pool(name="xin", bufs=8) as xinp,
            tc.tile_pool(name="gates", bufs=3) as gatep,
            tc.tile_pool(name="tmp", bufs=4) as tmpp,
            tc.tile_pool(name="psum", bufs=4, space="PSUM") as psump,
            tc.tile_pool(name="outs", bufs=3) as outsp,
        ):
            # weights via the gpsimd DMA queue; x stream owns the sync queue
            wih8 = constp.tile([128, 2, 1024], FP8, tag="wih8")
            bias = constp.tile([128, 8], F32, tag="bias")
            scr = constp.tile([128, 1], F32, tag="scr")
            nc.gpsimd.dma_start(wih8[:], wih8_d[:])
            nc.gpsimd.dma_start(bias[:], bc_d[:])
            # dummy 1-elem sigmoid pulls the ACT table load into startup
            nc.scalar.activation(scr[:, 0:1], bias[:, 0:1], _SIG)
            whh8 = constp.tile([128, 2, 1024], FP8, tag="whh8")
            nc.gpsimd.dma_start(whh8[:], whh8_d[:])
            w_x_b = constp.tile([128, 1024], BF16, tag="wxb")
            w_h0_b = constp.tile([128, 1024], BF16, tag="wh0b")
            w_h1_b = constp.tile([128, 1024], BF16, tag="wh1b")
            nc.gpsimd.dma_start(w_x_b[:], wlb_d[0])
            nc.gpsimd.dma_start(w_h0_b[:], wlb_d[1])
            nc.gpsimd.dma_start(w_h1_b[:], wlb_d[2])
            w_o = []
            for k in range(3):
                t_ = constp.tile([128, 256], BF16, tag=f"wo{k}")
                nc.gpsimd.dma_start(t_[:], wo_d[k])
                w_o.append(t_)
            h_b = constp.tile([128, 2, CH], BF16, tag="hb")
            bct8 = constp.tile([8, 128], BF16, tag="bct8")
            bct4b = constp.tile([4, 128], BF16, tag="bct4b")
            be8 = constp.tile([8, 8, 64], BF16, tag="be8")
            be4 = constp.tile([4, 4, 128], BF16, tag="be4")
            nc.gpsimd.dma_start(bct8[:], bct8_d[:])
            nc.gpsimd.dma_start(bct4b[:], bct8_d[4:8])
            nc.gpsimd.dma_start(be8[:], be8_d[:])
            nc.gpsimd.dma_start(be4[:], be4_d[:])

            # h: fp8 DoubleRow pairs; c: f32; hf: bf16 final-h (projection)
            h8_t, c_t, hf_t = [], [], []
            for j in range(NCH):
                h8 = statep.tile([128, 2, CH], FP8, tag=f"h8{j}")
                ct = statep.tile([128, 2, CH], F32, tag=f"c{j}")
                hf = statep.tile([128, 2, CH], BF16, tag=f"hf{j}")
                h8_t.append(h8)
                c_t.append(ct)
                hf_t.append(hf)

            for t in range(T):
                m = int(Mt[t])
                o_t = int(off[t])
                m_next = int(Mt[t + 1]) if t + 1 < T else 0
                tail = t >= TSW
                if t == TSW:
                    # snapshot chunk-0 h into the bf16 tail copy: active
                    # columns from fp8 state, finished columns from hf
                    wc = min(CH, AC)
                    mA = int(Mt[TSW])
                    nc.vector.tensor_copy(h_b[:, :, :mA],
                                          h8_t[0][:, :, :mA])
                    if wc > mA:
                        nc.vector.tensor_copy(h_b[:, :, mA:wc],
                                              hf_t[0][:, :, mA:wc])
                for j0 in range(0, m, CH):
                    j = j0 // CH
                    w = min(CH, m - j0)
                    if tail:
                        xt = xinp.tile([128, CH], BF16, tag="xb")
                        ob = o_t - S_wide + j0
                        nc.sync.dma_start(xt[:, :w], xseqb_d[:, ob : ob + w])
                    else:
                        xt = xinp.tile([128, 2, CH], FP8, tag="x")
                        nc.sync.dma_start(
                            xt[:, :, :w],
                            xseq8_d[:, :, o_t + j0 : o_t + j0 + w])

                    G = gatep.tile([128, 8, CH], BF16, tag="G")
                    if tail and w <= 128:
                        # Deep tail: pack 4 or 8 gate blocks per PSUM bank;
                        # bias lands first via one delta-pattern matmul.
                        nb = 1 if w <= 64 else 2
                        bpb = 8 // nb
                        be = be8 if nb == 1 else be4
                        psv = []
                        psq = psump.tile([128, 2, CH], F32, tag="ps")
                        for b in range(nb):
                            ps = psq[:, b, :]
                            pv = ps.rearrange("p (k c) -> p k c", k=bpb)
                            psv.append(pv)
                            blt = bct8[0:bpb, :] if b == 0 else bct4b[:]
                            nc.tensor.matmul(ps[:, :], blt, be[:, :, :],
                                             start=True, stop=False,
                                             skip_group_check=True)
                            for k in range(bpb):
                                mi = b * bpb + k
                                sl = slice(mi * 128, (mi + 1) * 128)
                                last = k == bpb - 1
                                nc.tensor.matmul(pv[:, k, :w], w_x_b[:, sl],
                                                 xt[:, :w], start=False,
                                                 stop=False,
                                                 skip_group_check=True)
                                nc.tensor.matmul(pv[:, k, :w], w_h0_b[:, sl],
                                                 h_b[:, 0, :w], start=False,
                                                 stop=False,
                                                 skip_group_check=True)
                                nc.tensor.matmul(pv[:, k, :w], w_h1_b[:, sl],
                                                 h_b[:, 1, :w], start=False,
                                                 stop=last,
                                                 skip_group_check=True)
                        if nb == 1:
                            pv = psv[0]
                            nc.scalar.activation(G[:, 0:4, :w], pv[:, 0:4, :w], _SIG)
                            nc.scalar.activation(G[:, 4:6, :w], pv[:, 4:6, :w], _TANH)
                            nc.scalar.activation(G[:, 6:8, :w], pv[:, 6:8, :w], _SIG)
                        else:
                            nc.scalar.activation(G[:, 0:4, :w], psv[0][:, :, :w], _SIG)
                            nc.scalar.activation(G[:, 4:6, :w], psv[1][:, 0:2, :w], _TANH)
                            nc.scalar.activation(G[:, 6:8, :w], psv[1][:, 2:4, :w], _SIG)
                    elif tail:
                        for mi0 in (0, 4, 2, 6):
                            ps = psump.tile([128, 2, CH], F32, tag="ps")
                            for k in (0, 1):
                                mi = mi0 + k
                                sl = slice(mi * 128, (mi + 1) * 128)
                                nc.tensor.matmul(ps[:, k, :w], w_x_b[:, sl],
                                                 xt[:, :w], start=True,
                                                 stop=False)
                                nc.tensor.matmul(ps[:, k, :w], w_h0_b[:, sl],
                                                 h_b[:, 0, :w],
                                                 start=False, stop=False)
                                nc.tensor.matmul(ps[:, k, :w], w_h1_b[:, sl],
                                                 h_b[:, 1, :w],
                                                 start=False, stop=True)
                                nc.scalar.activation(G[:, mi, :w],
                                                     ps[:, k, :w],
                                                     _GATE_FUNC[mi],
                                                     bias=bias[:, mi : mi + 1])
                    else:
                        # wide: two fp8 DoubleRow matmuls per gate block; the
                        # bias rides the x matmul (slot1 row0), so each gate
                        # PAIR shares one unbiased ACT over a 2-bank tile
                        for mi0 in (0, 4, 2, 6):  # i, g, f, o pairs
                            if t == 0 and mi0 == 2:
                                continue  # f gate unused at step 0 (c0 = 0)
                            ps = psump.tile([128, 2, CH], F32, tag="ps")
                            for k in (0, 1):
                                mi = mi0 + k
                                sl = slice(mi * 128, (mi + 1) * 128)
                                nc.tensor.matmul(ps[:, k, :w], wih8[:, :, sl],
                                                 xt[:, :, :w], start=True,
                                                 stop=(t == 0), perf_mode=DRM)
                                if t > 0:
                                    nc.tensor.matmul(ps[:, k, :w],
                                                     whh8[:, :, sl],
                                                     h8_t[j][:, :, :w],
                                                     start=False, stop=True,
                                                     perf_mode=DRM)
                            nc.scalar.activation(G[:, mi0 : mi0 + 2, :w],
                                                 ps[:, :, :w],
                                                 _GATE_FUNC[mi0])

                    cv = c_t[j][:, :, :w] if not tail else c_t[0][:, :, :w]
                    hv = h_b[:, :, :w] if tail else h8_t[j][:, :, :w]
                    th = tmpp.tile([128, 2, CH], BF16, tag="th")
                    if t == 0:
                        nc.vector.tensor_mul(cv, G[:, 0:2, :w], G[:, 4:6, :w])
                    else:
                        t1 = tmpp.tile([128, 2, CH], BF16, tag="t1")
                        nc.vector.tensor_mul(t1[:, :, :w], G[:, 0:2, :w],
                                             G[:, 4:6, :w])
                        nc.vector.tensor_mul(cv, cv, G[:, 2:4, :w])
                        nc.vector.tensor_add(cv, cv, t1[:, :, :w])
                    nc.scalar.activation(th[:, :, :w], cv, _TANH)
                    nc.vector.tensor_mul(hv, G[:, 6:8, :w], th[:, :, :w])
                    if not tail:
                        # bf16 final h for columns whose lifetime ends at t
                        lo = max(m_next, j0)
                        hi = min(m, j0 + CH)
                        if lo < hi:
                            ll, hh = lo - j0, hi - j0
                            nc.vector.tensor_mul(hf_t[j][:, :, ll:hh],
                                                 G[:, 6:8, ll:hh],
                                                 th[:, :, ll:hh])
                        # keep the PE activity window busy (HAM)
                        psd = psump.tile([128, 2, CH], F32, tag="ps")
                        nc.tensor.matmul(psd[:, 0, :CH], w_x_b[:, 0:128],
                                         w_x_b[:, 0:CH], start=True,
                                         stop=True, skip_group_check=True)

                    if tail and m > 300:
                        # mid-tail steps are latency-bound too; keep PE warm
                        for _d in range(2):
                            psd = psump.tile([128, 2, CH], F32, tag="ps")
                            nc.tensor.matmul(psd[:, _d % 2, :CH],
                                             w_x_b[:, 0:128],
                                             w_x_b[:, 0:CH],
                                             start=True, stop=True,
                                             skip_group_check=True)
                    if tail and m <= 300:
                        for _d in range(6):
                            psd = psump.tile([128, 2, CH], F32, tag="ps")
                            nc.tensor.matmul(psd[:, _d % 2, :CH],
                                             w_x_b[:, 0:128],
                                             w_x_b[:, 0:CH],
                                             start=True, stop=True,
                                             skip_group_check=True)

                # projections for finished chunks
                for j in range(NCH):
                    if proj_at[j] != t:
                        continue
                    j0 = j * CH
                    w = min(CH, AC - j0)
                    xo = xinp.tile([128, CH], BF16, tag="xo")
                    nc.sync.dma_start(xo[:, :w], xown_d[:, j0 : j0 + w])
                    if j == 0:
                        ph0, ph1 = h_b[:, 0, :w], h_b[:, 1, :w]
                    else:
                        ph0 = hf_t[j][:, 0, :w]
                        ph1 = hf_t[j][:, 1, :w]
                    psj = psump.tile([128, 2, CH], F32, tag="ps")
                    for mb in range(2):
                        ps = psj[:, mb, :]
                        sl = slice(mb * 128, (mb + 1) * 128)
                        nc.tensor.matmul(ps[:, :w], w_o[0][:, sl], xo[:, :w],
                                         start=True, stop=False)
                        nc.tensor.matmul(ps[:, :w], w_o[1][:, sl], ph0,
                                         start=False, stop=False)
                        nc.tensor.matmul(ps[:, :w], w_o[2][:, sl], ph1,
                                         start=False, stop=True)
                        ot = outsp.tile([128, CH], F32, tag="ot")
                        nc.vector.tensor_copy(ot[:, :w], ps[:, :w])
                        nc.sync.dma_start(out_d[mb, :, j0 : j0 + w],
                                          ot[:, :w])

    nc.compile()
    return nc


# ------------------------------------------------------------------ kernel

def run(inputs, trace=False, mm_dt=None):
    """Full pipeline; returns (output [N, OUT], BassKernelResults, pp)."""
    input_matrix = np.asarray(inputs["input_matrix"], np.float32)
    adjacency = np.asarray(inputs["adjacency"])
    W_ih = np.asarray(inputs["W_ih"], np.float32)
    W_hh = np.asarray(inputs["W_hh"], np.float32)
    b_ih = np.asarray(inputs["b_ih"], np.float32)
    b_hh = np.asarray(inputs["b_hh"], np.float32)
    W_out = np.asarray(inputs["W_out"], np.float32)

    pp = _preprocess(input_matrix, adjacency)
    nc = build_program(pp["T"], pp["M"], pp["off"], pp["S"], pp["AC"],
                       pp["TSW"])
    in_maps = _make_in_maps(pp, W_ih, W_hh, b_ih, b_hh, W_out)
    res = run_bass_kernel_spmd(nc, in_maps, list(range(NCORES)), trace=trace)

    N = input_matrix.shape[0]
    out = np.zeros((N, OUT), np.float32)
    for c in range(NCORES):
        oc = np.asarray(res.results[c]["out"]).reshape(OUT, pp["AC"])
        cn = pp["col_node"][c]
        valid = cn >= 0
        out[cn[valid]] = oc[:, valid].T
        if len(pp["deg0"][c]):
            z = pp["deg0"][c]
            out[z] = input_matrix[z] @ W_out[:F]  # h = 0 for degree-0 nodes
    return out, res, pp


def kernel(**inputs) -> np.ndarray:
    out, _, _ = run(inputs, trace=False)
    return out


# revision 11
# speedup vs baseline: 1.1849x; 1.1849x over previous
"""Trainium2 Bass kernel: LSTM neighbor-sequence aggregator + projection.

Model (reference): for each node v, run an LSTM (H=256) over the features
(F=128) of the targets of v's outgoing edges (in original edge order), take
the hidden state at the last valid step, concat with v's own features, and
project with W_out ([F+H, OUT]).

Strategy (v3 = baseline pipeline + fp8-DoubleRow wide matmuls)
--------------------------------------------------------------
Host (numpy):
  * Edges sorted by src (stable) -> per-node neighbor id lists; nodes dealt
    round-robin by degree rank onto 8 cores; shared non-increasing schedule
    M_t places each node at a column whose lifetime equals its degree.
  * Wide-phase neighbor features packed as fp8e4 pairs [128, 2, S_wide]:
    slot0 = e4m3(x), slot1 = e4m3(x - slot0) (residual -> x at ~2x mantissa).

Device (identical SPMD program, 8 cores):
  * Wide steps (M_t > 512): per 512-column chunk and gate block, TWO fp8
    DoubleRow matmuls (vs three f32r): W_ih pairs contract (x_hi, x_lo),
    W_hh pairs contract (h0, h1) with h stored as fp8 [128, 2, CH] --
    exactly the DoubleRow rhs layout.  PSUM/ACT/DVE pipeline is the proven
    per-bank structure: one PSUM bank per gate block (8 rotating tiles),
    per-block sigmoid/tanh ACT with the bias operand, DVE cell update.
    Filler matmuls after each chunk keep the PE activity window busy so
    HAM never re-throttles.
  * A bf16 copy of each column's final h is written for the narrow slice
    of columns ending at each step; projections read it (fp8 h would cost
    ~2e-3 extra error).
  * Tail steps (M_t <= 512): unchanged baseline bf16 machinery (packed
    deep-tail banks, delta-pattern bias matmuls, grouped ACTs, fillers).
"""

import math
import os
import sys

for _p in (
    "/opt/trn_rl_repo",
    "/root/.axon_site",
    "/root/.axon_site/_ro/trn_rl_repo",
    "/root/.axon_site/_ro/pypackages",
):
    if os.path.isdir(_p) and _p not in sys.path:
        sys.path.append(_p)

import numpy as np

import concourse.bass as bass
import concourse.tile as tile
from concourse import bacc, mybir
from concourse.bass_utils import run_bass_kernel_spmd

NCORES = 8
F, H, OUT = 128, 256, 256
CH = 512  # chunk width (matmul free dim; one fp32 PSUM bank)

F32 = mybir.dt.float32
BF16 = mybir.dt.bfloat16
FP8 = mybir.dt.float8e4
DRM = mybir.MatmulPerfMode.DoubleRow

_SIG = mybir.ActivationFunctionType.Sigmoid
_TANH = mybir.ActivationFunctionType.Tanh
# gate blocks (PyTorch order i,f,g,o; two 128-row blocks each)
_GATE_FUNC = [_SIG, _SIG, _SIG, _SIG, _TANH, _TANH, _SIG, _SIG]
# emit order: i,g first (DVE needs i*g), then f, then o
_MI_ORDER = [0, 1, 4, 5, 2, 3, 6, 7]


# ---------------------------------------------------------------- host side

def _preprocess(input_matrix, adjacency):
    """Partition nodes, build shared schedule + packed per-core inputs."""
    N = input_matrix.shape[0]
    src, trg = adjacency[0], adjacency[1]

    order = np.argsort(src, kind="stable")
    trg_s = trg[order]
    counts = np.bincount(src, minlength=N).astype(np.int64)
    offsets = np.zeros(N + 1, np.int64)
    np.cumsum(counts, out=offsets[1:])

    rank_order = np.argsort(-counts, kind="stable")
    core_nodes = [rank_order[c::NCORES] for c in range(NCORES)]
    deg_c = [counts[cn] for cn in core_nodes]

    T = int(counts.max())
    cnt = np.zeros((NCORES, T + 1), np.int64)
    for c in range(NCORES):
        h = np.bincount(deg_c[c], minlength=T + 1)
        cs = np.cumsum(h)
        cnt[c, :] = len(deg_c[c]) - cs[: T + 1]
    D = np.max(cnt[:, :-1] - cnt[:, 1:], axis=0)  # D[d-1] for d=1..T
    M = np.zeros(T + 1, np.int64)
    for t in range(T - 1, -1, -1):
        M[t] = -(-(M[t + 1] + D[t]) // 4) * 4

    ALL_COL = int(M[0])
    col_node = []
    deg0 = []
    for c in range(NCORES):
        cn = np.full(ALL_COL, -1, np.int64)
        for d in range(T, 0, -1):
            s0 = int(cnt[c, d])
            k = int(cnt[c, d - 1]) - s0
            if k:
                cn[int(M[d]) : int(M[d]) + k] = core_nodes[c][s0 : s0 + k]
        deg0.append(core_nodes[c][deg_c[c] == 0])  # handled on host
        col_node.append(cn)

    Mt = M[:-1]
    off = np.zeros(T + 1, np.int64)
    np.cumsum(Mt, out=off[1:])
    S = int(off[T])

    TSW = next((t for t in range(1, T) if int(Mt[t]) <= CH), T)
    S_wide = int(off[TSW])
    S_tail = S - S_wide

    e4 = mybir.dt.np(FP8)
    bf = mybir.dt.np(BF16)
    im32 = np.ascontiguousarray(input_matrix, np.float32)
    x_hi = im32.astype(e4)
    x_lo = (im32 - x_hi.astype(np.float32)).astype(e4)

    xseq8, xseqb, xown = [], [], []
    for c in range(NCORES):
        cn = col_node[c]
        xs8 = np.zeros((S_wide, 2, F), e4)
        xs8[:, 1, 0] = 1.0  # bias channel rides slot1 row0
        xsb = np.zeros((S_tail, F), np.float32)
        for t in range(T):
            m = int(Mt[t])
            colnodes = cn[:m]
            valid = colnodes >= 0
            vnodes = colnodes[valid]
            nbr = trg_s[offsets[vnodes] + t]
            if t < TSW:
                o = int(off[t])
                blk = xs8[o : o + m]
                tmp = blk[valid]
                tmp[:, 0, :] = x_hi[nbr]
                tmp[:, 1, 1:] = x_lo[nbr][:, 1:]  # row0 = bias channel
                blk[valid] = tmp
            else:
                o = int(off[t] - S_wide)
                xsb[o : o + m][valid] = im32[nbr]
        xseq8.append(np.ascontiguousarray(xs8.transpose(2, 1, 0)))
        xseqb.append(np.ascontiguousarray(xsb.T.astype(bf)))
        xo = np.zeros((ALL_COL, F), np.float32)
        valid = cn >= 0
        xo[valid] = im32[cn[valid]]
        xown.append(np.ascontiguousarray(xo.T.astype(bf)))

    return dict(T=T, M=Mt, off=off, S=S, AC=ALL_COL, TSW=TSW,
                xseq8=xseq8, xseqb=xseqb, xown=xown,
                col_node=col_node, deg0=deg0)


def _make_in_maps(pp, W_ih, W_hh, b_ih, b_hh, W_out):
    e4 = mybir.dt.np(FP8)
    bf = mybir.dt.np(BF16)
    Wi = np.ascontiguousarray(W_ih).astype(np.float32)   # [4H, F]
    Wh = np.ascontiguousarray(W_hh).astype(np.float32)   # [4H, H]
    bc = (b_ih + b_hh).astype(np.float32)                # [4H]

    Wi8 = Wi.T.astype(e4)                                # [F, 4H]
    wih8 = np.zeros((F, 2, 4 * H), e4)
    wih8[:, 0, :] = Wi8
    wih8[1:, 1, :] = Wi8[1:]     # slot1 pairs the x residual with the same W
    wih8[0, 1, :] = bc.astype(e4)  # ... except row0, which delivers the bias
    whh8 = np.zeros((128, 2, 4 * H), e4)
    whh8[:, 0, :] = Wh.T[0:128].astype(e4)
    whh8[:, 1, :] = Wh.T[128:256].astype(e4)

    wlb = np.stack([
        np.ascontiguousarray(Wi.T),
        np.ascontiguousarray(Wh.T[:128]),
        np.ascontiguousarray(Wh.T[128:]),
    ]).astype(bf)
    wo = np.stack([W_out[0:128], W_out[128:256], W_out[256:384]]).astype(bf)
    bcm = np.ascontiguousarray(bc.reshape(8, 128).T)     # [128, 8]
    bct8 = bcm.T.astype(bf)                              # [8, 128]
    be8 = np.zeros((8, 8, 64), np.float32)
    be8[np.arange(8), np.arange(8), :] = 1.0
    be4 = np.zeros((4, 4, 128), np.float32)
    be4[np.arange(4), np.arange(4), :] = 1.0

    maps = []
    for c in range(NCORES):
        maps.append({
            "xseq8": pp["xseq8"][c], "xseqb": pp["xseqb"][c],
            "xown": pp["xown"][c],
            "wih8": wih8, "whh8": whh8, "wlb": wlb, "wo": wo,
            "bc": bcm, "bct8": bct8,
            "be8": be8.astype(bf), "be4": be4.astype(bf),
        })
    return maps


# ------------------------------------------------------------- bass program

def build_program(T, Mt, off, S, AC, TSW):
    nc = bacc.Bacc("TRN2", target_bir_lowering=False, debug=False,
                   enable_asserts=False)

    S_wide = int(off[TSW])
    S_tail = int(off[T]) - S_wide

    xseq8_d = nc.declare_dram_parameter("xseq8", [128, 2, S_wide], FP8,
                                        isOutput=False)
    xseqb_d = nc.declare_dram_parameter("xseqb", [128, max(S_tail, 1)], BF16,
                                        isOutput=False)
    xown_d = nc.declare_dram_parameter("xown", [128, AC], BF16, isOutput=False)
    wih8_d = nc.declare_dram_parameter("wih8", [128, 2, 1024], FP8,
                                       isOutput=False)
    whh8_d = nc.declare_dram_parameter("whh8", [128, 2, 1024], FP8,
                                       isOutput=False)
    wlb_d = nc.declare_dram_parameter("wlb", [3, 128, 1024], BF16,
                                      isOutput=False)
    wo_d = nc.declare_dram_parameter("wo", [3, 128, 256], BF16, isOutput=False)
    bc_d = nc.declare_dram_parameter("bc", [128, 8], F32, isOutput=False)
    bct8_d = nc.declare_dram_parameter("bct8", [8, 128], BF16, isOutput=False)
    be8_d = nc.declare_dram_parameter("be8", [8, 8, 64], BF16, isOutput=False)
    be4_d = nc.declare_dram_parameter("be4", [4, 4, 128], BF16, isOutput=False)
    out_d = nc.declare_dram_parameter("out", [2, 128, AC], F32, isOutput=True)

    NCH = math.ceil(AC / CH)
    last_touch = [max(t for t in range(T) if Mt[t] > j * CH)
                  for j in range(NCH)]
    proj_at = [T - 1] + [min(max(last_touch[j], TSW + 3 * j), T - 2)
                         for j in range(1, NCH)]

    with tile.TileContext(nc) as tc:
        with (
            tc.tile_pool(name="const", bufs=1) as constp,
            tc.tile_pool(name="state", bufs=1) as statep,
            tc.tile_pool(name="xin", bufs=8) as xinp,
            tc.tile_pool(name="gates", bufs=3) as gatep,
            tc.tile_pool(name="tmp", bufs=4) as tmpp,
            tc.tile_pool(name="psum", bufs=4, space="PSUM") as psump,
            tc.tile_pool(name="outs", bufs=3) as outsp,
        ):
            # weights via the gpsimd DMA queue; x stream owns the sync queue
            wih8 = constp.tile([128, 2, 1024], FP8, tag="wih8")
            bias = constp.tile([128, 8], F32, tag="bias")
            scr = constp.tile([128, 1], F32, tag="scr")
            nc.gpsimd.dma_start(wih8[:], wih8_d[:])
            nc.gpsimd.dma_start(bias[:], bc_d[:])
            # dummy 1-elem sigmoid pulls the ACT table load into startup
            nc.scalar.activation(scr[:, 0:1], bias[:, 0:1], _SIG)
            whh8 = constp.tile([128, 2, 1024], FP8, tag="whh8")
            nc.gpsimd.dma_start(whh8[:], whh8_d[:])
            w_x_b = constp.tile([128, 1024], BF16, tag="wxb")
            w_h0_b = constp.tile([128, 1024], BF16, tag="wh0b")
            w_h1_b = constp.tile([128, 1024], BF16, tag="wh1b")
            nc.gpsimd.dma_start(w_x_b[:], wlb_d[0])
            nc.gpsimd.dma_start(w_h0_b[:], wlb_d[1])
            nc.gpsimd.dma_start(w_h1_b[:], wlb_d[2])
            w_o = []
            for k in range(3):
                t_ = constp.tile([128, 256], BF16, tag=f"wo{k}")
                nc.gpsimd.dma_start(t_[:], wo_d[k])
                w_o.append(t_)
            h_b = constp.tile([128, 2, CH], BF16, tag="hb")
            bct8 = constp.tile([8, 128], BF16, tag="bct8")
            bct4b = constp.tile([4, 128], BF16, tag="bct4b")
            be8 = constp.tile([8, 8, 64], BF16, tag="be8")
            be4 = constp.tile([4, 4, 128], BF16, tag="be4")
            nc.gpsimd.dma_start(bct8[:], bct8_d[:])
            nc.gpsimd.dma_start(bct4b[:], bct8_d[4:8])
            nc.gpsimd.dma_start(be8[:], be8_d[:])
            nc.gpsimd.dma_start(be4[:], be4_d[:])

            # h: fp8 DoubleRow pairs; c: f32; hf: bf16 final-h (projection)
            h8_t, c_t, hf_t = [], [], []
            for j in range(NCH):
                h8 = statep.tile([128, 2, CH], FP8, tag=f"h8{j}")
                ct = statep.tile([128, 2, CH], F32, tag=f"c{j}")
                hf = statep.tile([128, 2, CH], BF16, tag=f"hf{j}")
                h8_t.append(h8)
                c_t.append(ct)
                hf_t.append(hf)

            for t in range(T):
                m = int(Mt[t])
                o_t = int(off[t])
                m_next = int(Mt[t + 1]) if t + 1 < T else 0
                tail = t >= TSW
                if t == TSW:
                    # snapshot chunk-0 h into the bf16 tail copy: active
                    # columns from fp8 state, finished columns from hf
                    wc = min(CH, AC)
                    mA = int(Mt[TSW])
                    nc.vector.tensor_copy(h_b[:, :, :mA],
                                          h8_t[0][:, :, :mA])
                    if wc > mA:
                        nc.vector.tensor_copy(h_b[:, :, mA:wc],
                                              hf_t[0][:, :, mA:wc])
                for j0 in range(0, m, CH):
                    j = j0 // CH
                    w = min(CH, m - j0)
                    if tail:
                        xt = xinp.tile([128, CH], BF16, tag="xb")
                        ob = o_t - S_wide + j0
                        nc.sync.dma_start(xt[:, :w], xseqb_d[:, ob : ob + w])
                    else:
                        xt = xinp.tile([128, 2, CH], FP8, tag="x")
                        nc.sync.dma_start(
                            xt[:, :, :w],
                            xseq8_d[:, :, o_t + j0 : o_t + j0 + w])

                    G = gatep.tile([128, 8, CH], BF16, tag="G")
                    if tail and w <= 128:
                        # Deep tail: pack 4 or 8 gate blocks per PSUM bank;
                        # bias lands first via one delta-pattern matmul.
                        nb = 1 if w <= 64 else 2
                        bpb = 8 // nb
                        be = be8 if nb == 1 else be4
                        psv = []
                        psq = psump.tile([128, 2, CH], F32, tag="ps")
                        for b in range(nb):
                            ps = psq[:, b, :]
                            pv = ps.rearrange("p (k c) -> p k c", k=bpb)
                            psv.append(pv)
                            blt = bct8[0:bpb, :] if b == 0 else bct4b[:]
                            nc.tensor.matmul(ps[:, :], blt, be[:, :, :],
                                             start=True, stop=False,
                                             skip_group_check=True)
                            for k in range(bpb):
                                mi = b * bpb + k
                                sl = slice(mi * 128, (mi + 1) * 128)
                                last = k == bpb - 1
                                nc.tensor.matmul(pv[:, k, :w], w_x_b[:, sl],
                                                 xt[:, :w], start=False,
                                                 stop=False,
                                                 skip_group_check=True)
                                nc.tensor.matmul(pv[:, k, :w], w_h0_b[:, sl],
                                                 h_b[:, 0, :w], start=False,
                                                 stop=False,
                                                 skip_group_check=True)
                                nc.tensor.matmul(pv[:, k, :w], w_h1_b[:, sl],
                                                 h_b[:, 1, :w], start=False,
                                                 stop=last,
                                                 skip_group_check=True)
                        if nb == 1:
                            pv = psv[0]
                            nc.scalar.activation(G[:, 0:4, :w], pv[:, 0:4, :w], _SIG)
                            nc.scalar.activation(G[:, 4:6, :w], pv[:, 4:6, :w], _TANH)
                            nc.scalar.activation(G[:, 6:8, :w], pv[:, 6:8, :w], _SIG)
                        else:
                            nc.scalar.activation(G[:, 0:4, :w], psv[0][:, :, :w], _SIG)
                            nc.scalar.activation(G[:, 4:6, :w], psv[1][:, 0:2, :w], _TANH)
                            nc.scalar.activation(G[:, 6:8, :w], psv[1][:, 2:4, :w], _SIG)
                    elif tail:
                        for mi0 in (0, 4, 2, 6):
                            ps = psump.tile([128, 2, CH], F32, tag="ps")
                            for k in (0, 1):
                                mi = mi0 + k
                                sl = slice(mi * 128, (mi + 1) * 128)
                                nc.tensor.matmul(ps[:, k, :w], w_x_b[:, sl],
                                                 xt[:, :w], start=True,
                                                 stop=False)
                                nc.tensor.matmul(ps[:, k, :w], w_h0_b[:, sl],
                                                 h_b[:, 0, :w],
                                                 start=False, stop=False)
                                nc.tensor.matmul(ps[:, k, :w], w_h1_b[:, sl],
                                                 h_b[:, 1, :w],
                                                 start=False, stop=True)
                                nc.scalar.activation(G[:, mi, :w],
                                                     ps[:, k, :w],
                                                     _GATE_FUNC[mi],
                                                     bias=bias[:, mi : mi + 1])
                    else:
                        # wide: two fp8 DoubleRow matmuls per gate block; the
                        # bias rides the x matmul (slot1 row0), so each gate
                        # PAIR shares one unbiased ACT over a 2-bank tile
                        for mi0 in (0, 4, 2, 6):  # i, g, f, o pairs
                            if t == 0 and mi0 == 2:
                                continue  # f gate unused at step 0 (c0 = 0)
                            ps = psump.tile([128, 2, CH], F32, tag="ps")
                            for k in (0, 1):
                                mi = mi0 + k
                                sl = slice(mi * 128, (mi + 1) * 128)
                                nc.tensor.matmul(ps[:, k, :w], wih8[:, :, sl],
                                                 xt[:, :, :w], start=True,
                                                 stop=(t == 0), perf_mode=DRM)
                                if t > 0:
                                    nc.tensor.matmul(ps[:, k, :w],
                                                     whh8[:, :, sl],
                                                     h8_t[j][:, :, :w],
                                                     start=False, stop=True,
                                                     perf_mode=DRM)
                            nc.scalar.activation(G[:, mi0 : mi0 + 2, :w],
                                                 ps[:, :, :w],
                                                 _GATE_FUNC[mi0])

                    cv = c_t[j][:, :, :w] if not tail else c_t[0][:, :, :w]
                    hv = h_b[:, :, :w] if tail else h8_t[j][:, :, :w]
                    th = tmpp.tile([128, 2, CH], BF16, tag="th")
                    if t == 0:
                        nc.vector.tensor_mul(cv, G[:, 0:2, :w], G[:, 4:6, :w])
                    else:
                        t1 = tmpp.tile([128, 2, CH], BF16, tag="t1")
                        nc.vector.tensor_mul(t1[:, :, :w], G[:, 0:2, :w],
                                             G[:, 4:6, :w])
                        nc.vector.tensor_mul(cv, cv, G[:, 2:4, :w])
                        nc.vector.tensor_add(cv, cv, t1[:, :, :w])
                    nc.scalar.activation(th[:, :, :w], cv, _TANH)
                    nc.vector.tensor_mul(hv, G[:, 6:8, :w], th[:, :, :w])
                    if not tail:
                        # bf16 final h for columns whose lifetime ends at t
                        lo = max(m_next, j0)
                        hi = min(m, j0 + CH)
                        if lo < hi:
                            ll, hh = lo - j0, hi - j0
                            nc.vector.tensor_mul(hf_t[j][:, :, ll:hh],
                                                 G[:, 6:8, ll:hh],
                                                 th[:, :, ll:hh])
                        # keep the PE activity window busy (HAM)
                        psd = psump.tile([128, 2, CH], F32, tag="ps")
                        nc.tensor.matmul(psd[:, 0, :CH], w_x_b[:, 0:128],
                                         w_x_b[:, 0:CH], start=True,
                                         stop=True, skip_group_check=True)

                    if tail and m <= 300:
                        for _d in range(6):
                            psd = psump.tile([128, 2, CH], F32, tag="ps")
                            nc.tensor.matmul(psd[:, _d % 2, :CH],
                                             w_x_b[:, 0:128],
                                             w_x_b[:, 0:CH],
                                             start=True, stop=True,
                                             skip_group_check=True)

                # projections for finished chunks
                for j in range(NCH):
                    if proj_at[j] != t:
                        continue
                    j0 = j * CH
                    w = min(CH, AC - j0)
                    xo = xinp.tile([128, CH], BF16, tag="xo")
                    nc.sync.dma_start(xo[:, :w], xown_d[:, j0 : j0 + w])
                    if j == 0:
                        ph0, ph1 = h_b[:, 0, :w], h_b[:, 1, :w]
                    else:
                        ph0 = hf_t[j][:, 0, :w]
                        ph1 = hf_t[j][:, 1, :w]
                    psj = psump.tile([128, 2, CH], F32, tag="ps")
                    for mb in range(2):
                        ps = psj[:, mb, :]
                        sl = slice(mb * 128, (mb + 1) * 128)
                        nc.tensor.matmul(ps[:, :w], w_o[0][:, sl], xo[:, :w],
                                         start=True, stop=False)
                        nc.tensor.matmul(ps[:, :w], w_o[1][:, sl], ph0,
                                         start=False, stop=False)
                        nc.tensor.matmul(ps[:, :w], w_o[2][:, sl], ph1,
                                         start=False, stop=True)
                        ot = outsp.tile([128, CH], F32, tag="ot")
                        nc.vector.tensor_copy(ot[:, :w], ps[:, :w])
                        nc.sync.dma_start(out_d[mb, :, j0 : j0 + w],
                                          ot[:, :w])

    nc.compile()
    return nc


# ------------------------------------------------------------------ kernel

def run(inputs, trace=False, mm_dt=None):
    """Full pipeline; returns (output [N, OUT], BassKernelResults, pp)."""
    input_matrix = np.asarray(inputs["input_matrix"], np.float32)
    adjacency = np.asarray(inputs["adjacency"])
    W_ih = np.asarray(inputs["W_ih"], np.float32)
    W_hh = np.asarray(inputs["W_hh"], np.float32)
    b_ih = np.asarray(inputs["b_ih"], np.float32)
    b_hh = np.asarray(inputs["b_hh"], np.float32)
    W_out = np.asarray(inputs["W_out"], np.float32)

    pp = _preprocess(input_matrix, adjacency)
    nc = build_program(pp["T"], pp["M"], pp["off"], pp["S"], pp["AC"],
                       pp["TSW"])
    in_maps = _make_in_maps(pp, W_ih, W_hh, b_ih, b_hh, W_out)
    res = run_bass_kernel_spmd(nc, in_maps, list(range(NCORES)), trace=trace)

    N = input_matrix.shape[0]
    out = np.zeros((N, OUT), np.float32)
    for c in range(NCORES):
        oc = np.asarray(res.results[c]["out"]).reshape(OUT, pp["AC"])
        cn = pp["col_node"][c]
        valid = cn >= 0
        out[cn[valid]] = oc[:, valid].T
        if len(pp["deg0"][c]):
            z = pp["deg0"][c]
            out[z] = input_matrix[z] @ W_out[:F]  # h = 0 for degree-0 nodes
    return out, res, pp


def kernel(**inputs) -> np.ndarray:
    out, _, _ = run(inputs, trace=False)
    return out
